# revision 1
# baseline (speedup 1.0000x reference)
"""DyGAN generator Bass kernel for 8 TRN2 NeuronCores.

Sharding: node dim N=16384 split across 8 cores (2048 each).  The LSTM/time
paths are replicated on every core; the per-step softmax statistics
(E_down = e @ W_down_shard.T and rowsum) are combined with a single
AllReduce of a [129, 256] buffer per step.

Device layouts are transposed ([feature, batch]) with the H=512 dim folded
into [128 partitions, 4*256].  All matmuls run as fp32r (TF32-class).
The softmax normalization (divide by rowsum) and output layout restoration
happen on the host.
"""
import numpy as np

import concourse.bacc as bacc
import concourse.mybir as mybir
import concourse.tile as tile
from concourse import bass_utils

NCORES = 8
B = 256
H = 512
HIN = 128
HT = 32
N = 16384
NLOC = N // NCORES    # 2048
RW = 16
KEEP = np.float32(0.8)

F32 = mybir.dt.float32
F32R = mybir.dt.float32r
AF = mybir.ActivationFunctionType
OP = mybir.AluOpType
AX = mybir.AxisListType

_cached = {}


def fold_h(m, nchunks):
    """[nchunks*128, X] -> [128, nchunks*X]; column block k holds rows of
    chunk k."""
    ch, x = m.shape
    assert ch == nchunks * 128
    return np.ascontiguousarray(
        m.reshape(nchunks, 128, x).transpose(1, 0, 2).reshape(128, nchunks * x)
    ).astype(np.float32)


def build():
    nc = bacc.Bacc(None, target_bir_lowering=False, debug=False,
                   num_devices=NCORES)

    def din(name, shape, dt=F32R):
        return nc.dram_tensor(name, list(shape), dt, kind="ExternalInput").ap()

    whh_d = din("whh", [128, 8192])
    wiva_d = din("wiva", [128, 2048])
    wivb_d = din("wivb", [33, 2048])
    wih0_d = din("wih0", [128, 2048])
    x0t_d = din("x0t", [128, 256])
    wup_d = din("wup", [128, 8192])
    wdn_d = din("wdn", [128, 2048])
    cones_d = din("cones", [128, 256])
    wtd_d = din("wtd", [128, 128])
    btd_d = din("btd", [32, 1], F32)
    wtp_d = din("wtp", [32, 1])
    wtu_d = din("wtu", [1, 32])
    btu_d = din("btu", [32, 1], F32)
    wpva_d = din("wpva", [128, 1])
    wpvb_d = din("wpvb", [33, 1])
    cat0_d = din("cat0", [33, 256])
    hx0_d = din("hx0", [128, 1024])
    cs0_d = din("cs0", [128, 1024], F32)
    egb_d = din("egb", [RW, 128, 4096], F32)
    dut_d = din("dut", [RW, 32, 256], F32)

    out_e = nc.dram_tensor("out_e", [RW, 128, 4096], F32,
                           kind="ExternalOutput").ap()
    out_ts = nc.dram_tensor("out_ts", [RW, 1, 256], F32,
                            kind="ExternalOutput").ap()
    out_pr = nc.dram_tensor("out_pr", [1, 256], F32,
                            kind="ExternalOutput").ap()

    with tile.TileContext(nc) as tc:
        with tc.tile_pool(name="const", bufs=1) as cst, \
             tc.tile_pool(name="st", bufs=2) as st, \
             tc.tile_pool(name="gact", bufs=4) as gct, \
             tc.tile_pool(name="wk", bufs=1) as wk, \
             tc.tile_pool(name="epool", bufs=1) as epool, \
             tc.tile_pool(name="egbp", bufs=3) as egbp, \
             tc.tile_pool(name="tiny", bufs=2) as tiny, \
             tc.tile_pool(name="big", bufs=6, space="PSUM") as bigp, \
             tc.tile_pool(name="accsp", bufs=2, space="PSUM") as accp, \
             tc.tile_pool(name="dram", bufs=2, space="DRAM") as dram:

            # ---- constants ----
            whh = cst.tile([128, 8192], F32R)
            nc.sync.dma_start(whh[:], whh_d)
            wup = cst.tile([128, 8192], F32R)
            nc.sync.dma_start(wup[:], wup_d)
            wiva = cst.tile([128, 2048], F32R)   # holds W_ih.T for step 0
            nc.sync.dma_start(wiva[:], wih0_d)
            wivb = cst.tile([33, 2048], F32R)
            nc.sync.dma_start(wivb[:], wivb_d)
            wdn = cst.tile([128, 2048], F32R)
            nc.sync.dma_start(wdn[:], wdn_d)
            cones = cst.tile([128, 256], F32R)
            nc.sync.dma_start(cones[:], cones_d)
            wtd = cst.tile([128, 128], F32R)
            nc.sync.dma_start(wtd[:], wtd_d)
            btd = cst.tile([32, 1], F32)
            nc.sync.dma_start(btd[:], btd_d)
            wtp = cst.tile([32, 1], F32R)
            nc.sync.dma_start(wtp[:], wtp_d)
            wtu = cst.tile([1, 32], F32R)
            nc.sync.dma_start(wtu[:], wtu_d)
            btu = cst.tile([32, 1], F32)
            nc.sync.dma_start(btu[:], btu_d)
            wpva = cst.tile([128, 1], F32R)
            nc.sync.dma_start(wpva[:], wpva_d)
            wpvb = cst.tile([33, 1], F32R)
            nc.sync.dma_start(wpvb[:], wpvb_d)
            cat2 = cst.tile([33, 256], F32R)     # rows 0:32 = Ht, row 32 = 1
            nc.sync.dma_start(cat2[:], cat0_d)

            # ---- state ----
            hx = st.tile([128, 1024], F32R, tag="hx")
            nc.sync.dma_start(hx[:], hx0_d)
            cs = st.tile([128, 1024], F32, tag="cs")
            nc.sync.dma_start(cs[:], cs0_d)
            hg = st.tile([128, 256], F32R, tag="hg")   # step 0: x0T
            nc.sync.dma_start(hg[:], x0t_d)
            lastt = None
            cc_out_prev = None

            # gate index: i=0 f=1 g=2 o=3; process o,i,g first, f deferred
            EARLY, LATE = [3, 0, 2], [1]

            def gate_whh_mms(pg, G, h, s):
                """Accumulate W_hh part for gate G, half h (j in 2h,2h+1)."""
                pg[(G, h)] = bigp.tile([128, 512], F32, tag="big",
                                       name=f"pg{G}{h}_{s}")
                for jj in range(2):
                    j = 2 * h + jj
                    t = 4 * G + j
                    sl = pg[(G, h)][:, jj * 256:(jj + 1) * 256]
                    for k in range(4):
                        nc.tensor.matmul(
                            sl,
                            whh[:, k * 2048 + 128 * t:k * 2048 + 128 * t + 128],
                            hx[:, k * 256:(k + 1) * 256],
                            start=(k == 0), stop=False)

            def gate_x_mms(pg, G, h, lhs_a, rhs_a):
                for jj in range(2):
                    j = 2 * h + jj
                    t = 4 * G + j
                    sl = pg[(G, h)][:, jj * 256:(jj + 1) * 256]
                    nc.tensor.matmul(sl, lhs_a[:, 128 * t:128 * t + 128],
                                     rhs_a[:], start=False, stop=False)
                    nc.tensor.matmul(sl, wivb[:, 128 * t:128 * t + 128],
                                     cat2[:], start=False, stop=True)

            for s in range(RW):
                pg = {}
                # 1) W_hh part for o,i,g — no AR dependency, overlaps the
                #    previous step's AllReduce
                for G in EARLY:
                    for h in range(2):
                        gate_whh_mms(pg, G, h, s)

                # 2) phase A: consume previous AllReduce -> hg
                if s > 0:
                    ar = cc_out_prev
                    arE = tiny.tile([128, 256], F32, tag="arE",
                                    name=f"arE_{s}")
                    nc.sync.dma_start(arE[:], ar[0:128, :])
                    rsum = tiny.tile([1, 256], F32, tag="rsum",
                                     name=f"rs_{s}")
                    nc.sync.dma_start(rsum[:], ar[128:129, :])
                    recip = tiny.tile([1, 256], F32R, tag="recip",
                                      name=f"recip_{s}")
                    nc.vector.reciprocal(recip[:], rsum[:])
                    bc = accp.tile([128, 512], F32, tag="sp",
                                   name=f"bc_{s}")
                    nc.tensor.matmul(bc[:, 0:256], cones[0:1, 0:128],
                                     recip[:], start=True, stop=True)
                    hg_new = st.tile([128, 256], F32R, tag="hg",
                                     name=f"hg_{s}")
                    nc.vector.tensor_tensor(hg_new[:], arE[:], bc[:, 0:256],
                                            OP.mult)
                    hg = hg_new

                # 3) x parts (wait on hg) + deferred f gate
                xa = wiva
                for G in EARLY:
                    for h in range(2):
                        gate_x_mms(pg, G, h, xa, hg)
                for G in LATE:
                    for h in range(2):
                        gate_whh_mms(pg, G, h, s)
                        gate_x_mms(pg, G, h, xa, hg)
                if s == 0:
                    nc.sync.dma_start(wiva[:], wiva_d)  # real WivA for s>=1

                # 4) gate activations (tanh table only)
                def gact(name, G, scale):
                    tl = gct.tile([128, 1024], F32, tag=name[:2],
                                  name=f"{name}_{s}")
                    for h in range(2):
                        nc.scalar.activation(tl[:, h * 512:(h + 1) * 512],
                                             pg[(G, h)][:], AF.Tanh,
                                             scale=scale)
                    return tl

                so = gact("so", 3, 0.5)
                si = gact("si", 0, 0.5)
                tg = gact("tg", 2, 1.0)
                sf = gact("sf", 1, 0.5)

                # 5) LSTM pointwise
                a2 = wk.tile([128, 1024], F32, tag="a2", name=f"a2_{s}")
                nc.vector.scalar_tensor_tensor(a2[:], si[:], 1.0, tg[:],
                                               OP.add, OP.mult)
                a1 = wk.tile([128, 1024], F32, tag="a1", name=f"a1_{s}")
                nc.vector.scalar_tensor_tensor(a1[:], sf[:], 1.0, cs[:],
                                               OP.add, OP.mult)
                cs_new = st.tile([128, 1024], F32, tag="cs",
                                 name=f"cs_{s + 1}")
                nc.vector.scalar_tensor_tensor(cs_new[:], a1[:], 0.5, a2[:],
                                               OP.mult, OP.add)
                tc2 = wk.tile([128, 1024], F32, tag="tc2", name=f"tc2_{s}")
                nc.scalar.activation(tc2[:], cs_new[:], AF.Tanh, scale=0.5)
                hx_new = st.tile([128, 1024], F32R, tag="hx",
                                 name=f"hx_{s + 1}")
                nc.vector.scalar_tensor_tensor(hx_new[:], so[:], 1.0, tc2[:],
                                               OP.add, OP.mult)
                cs = cs_new
                hx = hx_new

                # 6) p = W_up2 @ hx ; e = exp(p) * egb ; partial sums
                e_sb = epool.tile([128, 4096], F32R, tag="e", name=f"e_{s}")
                acc = accp.tile([128, 512], F32, tag="sp", name=f"acc_{s}")
                for m in range(8):            # halves: n-tiles 2m, 2m+1
                    pq = bigp.tile([128, 512], F32, tag="big",
                                   name=f"pq{m}_{s}")
                    for jj in range(2):
                        t = 2 * m + jj
                        sl = pq[:, jj * 256:(jj + 1) * 256]
                        for k in range(4):
                            nc.tensor.matmul(
                                sl,
                                wup[:, k * 2048 + 128 * t:
                                    k * 2048 + 128 * t + 128],
                                hx[:, k * 256:(k + 1) * 256],
                                start=(k == 0), stop=(k == 3))
                    ep = gct.tile([128, 512], F32, tag="ep",
                                  name=f"ep{m}_{s}", bufs=3)
                    nc.scalar.activation(ep[:], pq[:], AF.Exp)
                    if m % 2 == 0:
                        eg = egbp.tile([128, 1024], F32, tag="egb",
                                       name=f"eg{m // 2}_{s}")
                        nc.sync.dma_start(
                            eg[:], egb_d[s][:, (m // 2) * 1024:
                                            (m // 2) * 1024 + 1024])
                    nc.vector.tensor_tensor(
                        e_sb[:, m * 512:(m + 1) * 512], ep[:],
                        eg[:, (m % 2) * 512:(m % 2) * 512 + 512], OP.mult)
                    for jj in range(2):
                        t = 2 * m + jj
                        ech = e_sb[:, t * 256:(t + 1) * 256]
                        nc.tensor.matmul(acc[:, 0:256],
                                         wdn[:, t * 128:t * 128 + 128],
                                         ech, start=(t == 0), stop=(t == 15))
                        nc.tensor.matmul(acc[0:1, 256:512], cones[:, 0:1],
                                         ech, start=(t == 0), stop=(t == 15))
                nc.sync.dma_start(out_e[s][:, 0:2048], e_sb[:, 0:2048])
                nc.sync.dma_start(out_e[s][:, 2048:4096], e_sb[:, 2048:4096])

                # 7) AllReduce of [E_down | rowsum]
                arin_e = tiny.tile([128, 256], F32, tag="arin_e",
                                   name=f"arin_e_{s}")
                nc.scalar.activation(arin_e[:], acc[:, 0:256], AF.Copy)
                arin_r = tiny.tile([1, 256], F32, tag="arin_r",
                                   name=f"arin_r_{s}")
                nc.scalar.activation(arin_r[:], acc[0:1, 256:512], AF.Copy)
                cc_in = dram.tile([129, 256], F32, tag="ci", name=f"ci_{s}")
                nc.sync.dma_start(cc_in[0:128, :], arin_e[:])
                nc.sync.dma_start(cc_in[128:129, :], arin_r[:])
                cc_out = dram.tile([129, 256], F32, tag="co",
                                   addr_space="Shared", name=f"co_{s}")
                nc.gpsimd.collective_compute(
                    "AllReduce", OP.add,
                    replica_groups=[list(range(NCORES))],
                    ins=[cc_in[:].opt()], outs=[cc_out[:].opt()])
                cc_out_prev = cc_out

                # 8) time path — overlaps the AllReduce
                tdp = accp.tile([128, 512], F32, tag="sp", name=f"tdp_{s}")
                for k in range(4):
                    nc.tensor.matmul(tdp[0:32, 0:256],
                                     wtd[:, k * 32:(k + 1) * 32],
                                     hx[:, k * 256:(k + 1) * 256],
                                     start=(k == 0), stop=(k == 3))
                tdm = tiny.tile([32, 256], F32, tag="tdm", name=f"tdm_{s}")
                nc.scalar.activation(tdm[:], tdp[0:32, 0:256], AF.Tanh,
                                     bias=btd[:])
                dut_s = tiny.tile([32, 256], F32, tag="dut", name=f"dut_{s}")
                nc.sync.dma_start(dut_s[:], dut_d[s])
                tdm2 = tiny.tile([32, 256], F32R, tag="tdm2",
                                 name=f"tdm2_{s}")
                nc.vector.scalar_tensor_tensor(tdm2[:], dut_s[:], float(KEEP),
                                               tdm[:], OP.is_lt, OP.mult)
                tp = accp.tile([128, 512], F32, tag="sp", name=f"tp_{s}")
                nc.tensor.matmul(tp[0:1, 0:256], wtp[:], tdm2[:],
                                 start=True, stop=True)
                traw = tiny.tile([1, 256], F32, tag="traw", name=f"traw_{s}")
                nc.vector.tensor_scalar_min(traw[:], tp[0:1, 0:256], 1.0)
                mn = tiny.tile([1, 1], F32, tag="mn", name=f"mn_{s}")
                nc.vector.tensor_reduce(mn[:], traw[:], AX.X, OP.min)
                flag = tiny.tile([1, 1], F32, tag="flag", name=f"flag_{s}")
                nc.vector.tensor_single_scalar(flag[:], mn[:], 0.1, OP.is_lt)
                delta = tiny.tile([1, 1], F32, tag="delta", name=f"delta_{s}")
                nc.vector.tensor_tensor(delta[:], mn[:], flag[:], OP.mult)
                t2 = tiny.tile([1, 256], F32, tag="t2", name=f"t2_{s}")
                nc.vector.tensor_scalar_sub(t2[:], traw[:], delta[:])
                mx = tiny.tile([1, 1], F32, tag="mx", name=f"mx_{s}")
                nc.vector.tensor_reduce(mx[:], t2[:], AX.X, OP.max)
                r1 = tiny.tile([1, 1], F32, tag="r1", name=f"r1_{s}")
                nc.vector.reciprocal(r1[:], mx[:])
                gtf = tiny.tile([1, 1], F32, tag="gtf", name=f"gtf_{s}")
                nc.vector.tensor_single_scalar(gtf[:], mx[:], 1.0, OP.is_gt)
                fac = tiny.tile([1, 1], F32, tag="fac", name=f"fac_{s}")
                nc.vector.scalar_tensor_tensor(fac[:], r1[:], -1.0, gtf[:],
                                               OP.add, OP.mult)
                t3 = tiny.tile([1, 256], F32, tag="t3", name=f"t3_{s}")
                nc.vector.scalar_tensor_tensor(t3[:], t2[:], fac[:], t2[:],
                                               OP.mult, OP.add)
                t5 = tiny.tile([1, 256], F32R, tag="t5", name=f"t5_{s}")
                if lastt is None:
                    # last_t = 0 and t >= 0 after the shift: only min vs 1
                    nc.vector.tensor_scalar_min(t5[:], t3[:], 1.0)
                else:
                    t4 = tiny.tile([1, 256], F32, tag="t4", name=f"t4_{s}")
                    nc.vector.tensor_tensor(t4[:], t3[:], lastt[:], OP.max)
                    nc.vector.tensor_scalar_min(t5[:], t4[:], 1.0)
                lastt = t5
                nc.sync.dma_start(out_ts[s], t5[:])
                htp = accp.tile([128, 512], F32, tag="sp", name=f"htp_{s}")
                nc.tensor.matmul(htp[0:32, 0:256], wtu[:], t5[:],
                                 start=True, stop=True)
                nc.scalar.activation(cat2[0:32, :], htp[0:32, 0:256],
                                     AF.Copy, bias=btu[:])

            # epilogue: final Hg from last AllReduce, then prob
            ar = cc_out_prev
            arE = tiny.tile([128, 256], F32, tag="arE", name="arE_f")
            nc.sync.dma_start(arE[:], ar[0:128, :])
            rsum = tiny.tile([1, 256], F32, tag="rsum", name="rs_f")
            nc.sync.dma_start(rsum[:], ar[128:129, :])
            recip = tiny.tile([1, 256], F32R, tag="recip", name="recip_f")
            nc.vector.reciprocal(recip[:], rsum[:])
            bc = accp.tile([128, 512], F32, tag="sp", name="bc_f")
            nc.tensor.matmul(bc[:, 0:256], cones[0:1, 0:128], recip[:],
                             start=True, stop=True)
            hg_f = st.tile([128, 256], F32R, tag="hg", name="hg_f")
            nc.vector.tensor_tensor(hg_f[:], arE[:], bc[:, 0:256], OP.mult)
            prp = accp.tile([128, 512], F32, tag="sp", name="prp")
            nc.tensor.matmul(prp[0:1, 0:256], wpva[:], hg_f[:], start=True,
                             stop=False)
            nc.tensor.matmul(prp[0:1, 0:256], wpvb[:], cat2[:], start=False,
                             stop=True)
            prout = tiny.tile([1, 256], F32, tag="prout", name="prout")
            nc.scalar.activation(prout[:], prp[0:1, 0:256], AF.Copy)
            nc.sync.dma_start(out_pr, prout[:])

    nc.compile()
    return nc


def prep_inputs(inputs):
    """Host-side preparation of all per-core DRAM parameters."""
    f32 = np.float32
    W_ih = np.asarray(inputs["W_ih"], f32)
    W_hh = np.asarray(inputs["W_hh"], f32)
    b_ih = np.asarray(inputs["b_ih"], f32)
    b_hh = np.asarray(inputs["b_hh"], f32)
    W_up = np.asarray(inputs["W_up"], f32)
    b_up = np.asarray(inputs["b_up"], f32)
    W_down = np.asarray(inputs["W_down"], f32)
    W_vt = np.asarray(inputs["W_vt"], f32)
    W_prob = np.asarray(inputs["W_prob"], f32)
    Wt_down = np.asarray(inputs["Wt_down"], f32)
    bt_down = np.asarray(inputs["bt_down"], f32)
    Wt_pred = np.asarray(inputs["Wt_pred"], f32)
    Wt_up = np.asarray(inputs["Wt_up"], f32)
    bt_up = np.asarray(inputs["bt_up"], f32)
    gumbel = np.asarray(inputs["gumbel"], f32)
    drop_u = np.asarray(inputs["drop_u"], f32)
    latent = np.asarray(inputs["latent"], f32)
    inputs0 = np.asarray(inputs["inputs0"], f32)

    lat = np.tanh(latent @ np.asarray(inputs["Wc"], f32).T
                  + np.asarray(inputs["bc"], f32))
    s_ = np.tanh(lat @ np.asarray(inputs["Ws"], f32).T
                 + np.asarray(inputs["bs"], f32))
    h0 = np.tanh(s_ @ np.asarray(inputs["Wh"], f32).T
                 + np.asarray(inputs["bh"], f32))
    c0 = np.tanh(s_ @ np.asarray(inputs["Wcc"], f32).T
                 + np.asarray(inputs["bcc"], f32))

    W_ihvt = (W_ih @ W_vt).astype(f32)            # [2048, 160]
    bias_g = (b_ih + b_hh).astype(f32)            # [2048]
    W_pvt = (W_prob @ W_vt).astype(f32)           # [1, 160]

    shared = {
        "whh": fold_h(W_hh.T * 0.5, 4),
        "wiva": np.ascontiguousarray(W_ihvt[:, :128].T),
        "wivb": np.ascontiguousarray(
            np.concatenate([W_ihvt[:, 128:160].T, bias_g[None, :]], axis=0)),
        "wih0": np.ascontiguousarray(W_ih.T),
        "x0t": np.ascontiguousarray(inputs0.T),
        "cones": np.ones((128, 256), f32),
        "wtd": fold_h(Wt_down.T * 0.5, 4),
        "btd": np.ascontiguousarray(bt_down.reshape(32, 1)),
        "wtp": np.ascontiguousarray((Wt_pred / KEEP).T),
        "wtu": np.ascontiguousarray(Wt_up.T),
        "btu": np.ascontiguousarray(bt_up.reshape(32, 1)),
        "wpva": np.ascontiguousarray(W_pvt[:, :128].T),
        "wpvb": np.ascontiguousarray(
            np.concatenate([W_pvt[:, 128:160].T, np.zeros((1, 1), f32)],
                           axis=0)),
        "cat0": np.concatenate([np.zeros((32, 256), f32),
                                np.ones((1, 256), f32)], axis=0),
        "hx0": fold_h(2.0 * h0.T, 4),
        "cs0": fold_h(2.0 * c0.T, 4),
        "dut": np.ascontiguousarray(drop_u.transpose(0, 2, 1)),
    }

    G = np.exp(gumbel + b_up[None, None, :]).astype(f32)       # [16, 256, N]
    in_maps = []
    for c in range(NCORES):
        Wc_up = W_up[c * NLOC:(c + 1) * NLOC, :]
        Wc_dn = W_down[:, c * NLOC:(c + 1) * NLOC]
        Gc = G[:, :, c * NLOC:(c + 1) * NLOC]
        egb = np.ascontiguousarray(
            Gc.reshape(RW, 256, 16, 128).transpose(0, 3, 2, 1)
              .reshape(RW, 128, 4096))
        m = dict(shared)
        m["wup"] = fold_h(Wc_up.T * 0.5, 4)
        m["wdn"] = fold_h(np.ascontiguousarray(Wc_dn.T), 16)
        m["egb"] = egb
        in_maps.append(m)
    return in_maps


def assemble(results):
    """Gather per-core outputs into (rw, ts, prob)."""
    E = np.stack([results[c]["out_e"] for c in range(NCORES)])
    E = E.reshape(NCORES, RW, 128, 16, 256)
    # rw[b, s, c*2048 + t*128 + p] = E[c, s, p, t, b]
    rw = np.ascontiguousarray(E.transpose(4, 1, 0, 3, 2)).reshape(B, RW, N)
    rowsum = rw.sum(axis=2, dtype=np.float64)
    rw = (rw / rowsum[:, :, None]).astype(np.float32)
    ts = np.ascontiguousarray(
        results[0]["out_ts"][:, 0, :].T).reshape(B, RW, 1)
    prob = np.ascontiguousarray(results[0]["out_pr"].T)
    return rw, ts, prob


def get_nc():
    if "nc" not in _cached:
        _cached["nc"] = build()
    return _cached["nc"]


def kernel(**inputs):
    nc = get_nc()
    in_maps = prep_inputs(inputs)
    res = bass_utils.run_bass_kernel_spmd(nc, in_maps,
                                          core_ids=list(range(NCORES)))
    return assemble(res.results)


# revision 2
# speedup vs baseline: 1.0351x; 1.0351x over previous
"""DyGAN generator Bass kernel for 8 TRN2 NeuronCores.

Sharding: node dim N=16384 split across 8 cores (2048 each).  The LSTM/time
paths are replicated on every core; the per-step softmax statistics
(E_down = e @ W_down_shard.T and rowsum) are combined with a single
AllReduce of a [129, 256] buffer per step.

Device layouts are transposed ([feature, batch]) with the H=512 dim folded
into [128 partitions, 4*256].  All matmuls run as fp32r (TF32-class).
The softmax normalization (divide by rowsum) and output layout restoration
happen on the host.
"""
import numpy as np

import concourse.bacc as bacc
import concourse.mybir as mybir
import concourse.tile as tile
from concourse import bass_utils

NCORES = 8
B = 256
H = 512
HIN = 128
HT = 32
N = 16384
NLOC = N // NCORES    # 2048
RW = 16
KEEP = np.float32(0.8)

F32 = mybir.dt.float32
F32R = mybir.dt.float32r
AF = mybir.ActivationFunctionType
OP = mybir.AluOpType
AX = mybir.AxisListType

_cached = {}


def fold_h(m, nchunks):
    """[nchunks*128, X] -> [128, nchunks*X]; column block k holds rows of
    chunk k."""
    ch, x = m.shape
    assert ch == nchunks * 128
    return np.ascontiguousarray(
        m.reshape(nchunks, 128, x).transpose(1, 0, 2).reshape(128, nchunks * x)
    ).astype(np.float32)


def build():
    nc = bacc.Bacc(None, target_bir_lowering=False, debug=False,
                   num_devices=NCORES)

    def din(name, shape, dt=F32R):
        return nc.dram_tensor(name, list(shape), dt, kind="ExternalInput").ap()

    whh_d = din("whh", [128, 8192])
    wiva_d = din("wiva", [128, 2048])
    wivb_d = din("wivb", [33, 2048])
    wih0_d = din("wih0", [128, 2048])
    x0t_d = din("x0t", [128, 256])
    wup_d = din("wup", [128, 8192])
    wdn_d = din("wdn", [128, 2048])
    cones_d = din("cones", [128, 256])
    wtd_d = din("wtd", [128, 128])
    btd_d = din("btd", [32, 1], F32)
    wtp_d = din("wtp", [32, 1])
    wtu_d = din("wtu", [1, 32])
    btu_d = din("btu", [32, 1], F32)
    wpva_d = din("wpva", [128, 1])
    wpvb_d = din("wpvb", [33, 1])
    cat0_d = din("cat0", [33, 256])
    hx0_d = din("hx0", [128, 1024])
    cs0_d = din("cs0", [128, 1024], F32)
    egb_d = din("egb", [RW, 128, 4096], F32)
    dut_d = din("dut", [RW, 32, 256], F32)

    out_e = nc.dram_tensor("out_e", [RW, 128, 4096], F32,
                           kind="ExternalOutput").ap()
    out_ts = nc.dram_tensor("out_ts", [RW, 1, 256], F32,
                            kind="ExternalOutput").ap()
    out_pr = nc.dram_tensor("out_pr", [1, 256], F32,
                            kind="ExternalOutput").ap()

    with tile.TileContext(nc) as tc:
        with tc.tile_pool(name="const", bufs=1) as cst, \
             tc.tile_pool(name="st", bufs=2) as st, \
             tc.tile_pool(name="gact", bufs=4) as gct, \
             tc.tile_pool(name="wk", bufs=1) as wk, \
             tc.tile_pool(name="epool", bufs=1) as epool, \
             tc.tile_pool(name="egbp", bufs=3) as egbp, \
             tc.tile_pool(name="tiny", bufs=2) as tiny, \
             tc.tile_pool(name="big", bufs=7, space="PSUM") as bigp, \
             tc.tile_pool(name="accsp", bufs=1, space="PSUM") as accp, \
             tc.tile_pool(name="dram", bufs=2, space="DRAM") as dram:

            # ---- constants ----
            whh = cst.tile([128, 8192], F32R)
            nc.sync.dma_start(whh[:], whh_d)
            wup = cst.tile([128, 8192], F32R)
            nc.sync.dma_start(wup[:], wup_d)
            wiva = cst.tile([128, 2048], F32R)   # holds W_ih.T for step 0
            nc.sync.dma_start(wiva[:], wih0_d)
            wivb = cst.tile([33, 2048], F32R)
            nc.sync.dma_start(wivb[:], wivb_d)
            wdn = cst.tile([128, 2048], F32R)
            nc.sync.dma_start(wdn[:], wdn_d)
            cones = cst.tile([128, 256], F32R)
            nc.sync.dma_start(cones[:], cones_d)
            wtd = cst.tile([128, 128], F32R)
            nc.sync.dma_start(wtd[:], wtd_d)
            btd = cst.tile([32, 1], F32)
            nc.sync.dma_start(btd[:], btd_d)
            wtp = cst.tile([32, 1], F32R)
            nc.sync.dma_start(wtp[:], wtp_d)
            wtu = cst.tile([1, 32], F32R)
            nc.sync.dma_start(wtu[:], wtu_d)
            btu = cst.tile([32, 1], F32)
            nc.sync.dma_start(btu[:], btu_d)
            wpva = cst.tile([128, 1], F32R)
            nc.sync.dma_start(wpva[:], wpva_d)
            wpvb = cst.tile([33, 1], F32R)
            nc.sync.dma_start(wpvb[:], wpvb_d)
            cat2 = cst.tile([33, 256], F32R)     # rows 0:32 = Ht, row 32 = 1
            nc.sync.dma_start(cat2[:], cat0_d)

            # ---- state ----
            hx = st.tile([128, 1024], F32R, tag="hx")
            nc.sync.dma_start(hx[:], hx0_d)
            cs = st.tile([128, 1024], F32, tag="cs")
            nc.sync.dma_start(cs[:], cs0_d)
            hg = st.tile([128, 256], F32R, tag="hg")   # step 0: x0T
            nc.sync.dma_start(hg[:], x0t_d)
            lastt = None
            cc_out_prev = None

            # gate index: i=0 f=1 g=2 o=3; process o,i,g first, f deferred
            EARLY, LATE = [3, 0, 2], [1]

            def gate_whh_mms(pg, G, h, s):
                """Accumulate W_hh part for gate G, half h (j in 2h,2h+1)."""
                pg[(G, h)] = bigp.tile([128, 512], F32, tag="big",
                                       name=f"pg{G}{h}_{s}")
                for jj in range(2):
                    j = 2 * h + jj
                    t = 4 * G + j
                    sl = pg[(G, h)][:, jj * 256:(jj + 1) * 256]
                    for k in range(4):
                        nc.tensor.matmul(
                            sl,
                            whh[:, k * 2048 + 128 * t:k * 2048 + 128 * t + 128],
                            hx[:, k * 256:(k + 1) * 256],
                            start=(k == 0), stop=False)

            def gate_x_mms(pg, G, h, lhs_a, rhs_a):
                for jj in range(2):
                    j = 2 * h + jj
                    t = 4 * G + j
                    sl = pg[(G, h)][:, jj * 256:(jj + 1) * 256]
                    nc.tensor.matmul(sl, lhs_a[:, 128 * t:128 * t + 128],
                                     rhs_a[:], start=False, stop=False)
                    nc.tensor.matmul(sl, wivb[:, 128 * t:128 * t + 128],
                                     cat2[:], start=False, stop=True)

            for s in range(RW):
                pg = {}
                # 1) W_hh part for o,i,g — no AR dependency, overlaps the
                #    previous step's AllReduce
                for G in EARLY:
                    for h in range(2):
                        gate_whh_mms(pg, G, h, s)

                # 2) phase A: consume previous AllReduce -> hg
                if s > 0:
                    ar = cc_out_prev
                    arE = tiny.tile([128, 256], F32, tag="arE",
                                    name=f"arE_{s}")
                    nc.sync.dma_start(arE[:], ar[0:128, :])
                    rsum = tiny.tile([1, 256], F32, tag="rsum",
                                     name=f"rs_{s}")
                    nc.sync.dma_start(rsum[:], ar[128:129, :])
                    recip = tiny.tile([1, 256], F32R, tag="recip",
                                      name=f"recip_{s}")
                    nc.vector.reciprocal(recip[:], rsum[:])
                    bc = accp.tile([128, 512], F32, tag="sp",
                                   name=f"bc_{s}")
                    nc.tensor.matmul(bc[:, 0:256], cones[0:1, 0:128],
                                     recip[:], start=True, stop=True)
                    hg_new = st.tile([128, 256], F32R, tag="hg",
                                     name=f"hg_{s}")
                    nc.vector.tensor_tensor(hg_new[:], arE[:], bc[:, 0:256],
                                            OP.mult)
                    hg = hg_new

                # 3) x parts (wait on hg) + deferred f gate
                xa = wiva
                for G in EARLY:
                    for h in range(2):
                        gate_x_mms(pg, G, h, xa, hg)
                for G in LATE:
                    for h in range(2):
                        gate_whh_mms(pg, G, h, s)
                        gate_x_mms(pg, G, h, xa, hg)
                if s == 0:
                    nc.sync.dma_start(wiva[:], wiva_d)  # real WivA for s>=1

                # 4) gate activations (tanh table only)
                def gact(name, G, scale):
                    tl = gct.tile([128, 1024], F32, tag=name[:2],
                                  name=f"{name}_{s}")
                    for h in range(2):
                        nc.scalar.activation(tl[:, h * 512:(h + 1) * 512],
                                             pg[(G, h)][:], AF.Tanh,
                                             scale=scale)
                    return tl

                so = gact("so", 3, 0.5)
                si = gact("si", 0, 0.5)
                tg = gact("tg", 2, 1.0)
                sf = gact("sf", 1, 0.5)

                # 5) LSTM pointwise
                a2 = wk.tile([128, 1024], F32, tag="a2", name=f"a2_{s}")
                nc.vector.scalar_tensor_tensor(a2[:], si[:], 1.0, tg[:],
                                               OP.add, OP.mult)
                a1 = wk.tile([128, 1024], F32, tag="a1", name=f"a1_{s}")
                nc.vector.scalar_tensor_tensor(a1[:], sf[:], 1.0, cs[:],
                                               OP.add, OP.mult)
                cs_new = st.tile([128, 1024], F32, tag="cs",
                                 name=f"cs_{s + 1}")
                nc.vector.scalar_tensor_tensor(cs_new[:], a1[:], 0.5, a2[:],
                                               OP.mult, OP.add)
                tc2 = wk.tile([128, 1024], F32, tag="tc2", name=f"tc2_{s}")
                nc.scalar.activation(tc2[:], cs_new[:], AF.Tanh, scale=0.5)
                hx_new = st.tile([128, 1024], F32R, tag="hx",
                                 name=f"hx_{s + 1}")
                nc.vector.scalar_tensor_tensor(hx_new[:], so[:], 1.0, tc2[:],
                                               OP.add, OP.mult)
                cs = cs_new
                hx = hx_new

                # 6) p = W_up2 @ hx ; e = exp(p) * egb ; partial sums
                e_sb = epool.tile([128, 4096], F32R, tag="e", name=f"e_{s}")
                acc = accp.tile([128, 512], F32, tag="sp", name=f"acc_{s}")
                for m in range(8):            # halves: n-tiles 2m, 2m+1
                    pq = bigp.tile([128, 512], F32, tag="big",
                                   name=f"pq{m}_{s}")
                    for jj in range(2):
                        t = 2 * m + jj
                        sl = pq[:, jj * 256:(jj + 1) * 256]
                        for k in range(4):
                            nc.tensor.matmul(
                                sl,
                                wup[:, k * 2048 + 128 * t:
                                    k * 2048 + 128 * t + 128],
                                hx[:, k * 256:(k + 1) * 256],
                                start=(k == 0), stop=(k == 3))
                    ep = gct.tile([128, 512], F32, tag="ep",
                                  name=f"ep{m}_{s}", bufs=3)
                    nc.scalar.activation(ep[:], pq[:], AF.Exp)
                    if m % 2 == 0:
                        eg = egbp.tile([128, 1024], F32, tag="egb",
                                       name=f"eg{m // 2}_{s}")
                        nc.sync.dma_start(
                            eg[:], egb_d[s][:, (m // 2) * 1024:
                                            (m // 2) * 1024 + 1024])
                    nc.vector.tensor_tensor(
                        e_sb[:, m * 512:(m + 1) * 512], ep[:],
                        eg[:, (m % 2) * 512:(m % 2) * 512 + 512], OP.mult)
                    for jj in range(2):
                        t = 2 * m + jj
                        ech = e_sb[:, t * 256:(t + 1) * 256]
                        nc.tensor.matmul(acc[:, 0:256],
                                         wdn[:, t * 128:t * 128 + 128],
                                         ech, start=(t == 0), stop=(t == 15))
                        nc.tensor.matmul(acc[0:1, 256:512], cones[:, 0:1],
                                         ech, start=(t == 0), stop=(t == 15))
                nc.sync.dma_start(out_e[s][:, 0:2048], e_sb[:, 0:2048])
                nc.sync.dma_start(out_e[s][:, 2048:4096], e_sb[:, 2048:4096])

                # 7) AllReduce of [E_down | rowsum]
                arin_e = tiny.tile([128, 256], F32, tag="arin_e",
                                   name=f"arin_e_{s}")
                nc.scalar.activation(arin_e[:], acc[:, 0:256], AF.Copy)
                arin_r = tiny.tile([1, 256], F32, tag="arin_r",
                                   name=f"arin_r_{s}")
                nc.scalar.activation(arin_r[:], acc[0:1, 256:512], AF.Copy)
                cc_in = dram.tile([129, 256], F32, tag="ci", name=f"ci_{s}")
                nc.sync.dma_start(cc_in[0:128, :], arin_e[:])
                nc.sync.dma_start(cc_in[128:129, :], arin_r[:])
                cc_out = dram.tile([129, 256], F32, tag="co",
                                   addr_space="Shared", name=f"co_{s}")
                nc.gpsimd.collective_compute(
                    "AllReduce", OP.add,
                    replica_groups=[list(range(NCORES))],
                    ins=[cc_in[:].opt()], outs=[cc_out[:].opt()])
                cc_out_prev = cc_out

                # 8) time path — overlaps the AllReduce
                tdp = accp.tile([128, 512], F32, tag="sp", name=f"tdp_{s}")
                for k in range(4):
                    nc.tensor.matmul(tdp[0:32, 0:256],
                                     wtd[:, k * 32:(k + 1) * 32],
                                     hx[:, k * 256:(k + 1) * 256],
                                     start=(k == 0), stop=(k == 3))
                tdm = tiny.tile([32, 256], F32, tag="tdm", name=f"tdm_{s}")
                nc.scalar.activation(tdm[:], tdp[0:32, 0:256], AF.Tanh,
                                     bias=btd[:])
                dut_s = tiny.tile([32, 256], F32, tag="dut", name=f"dut_{s}")
                nc.sync.dma_start(dut_s[:], dut_d[s])
                tdm2 = tiny.tile([32, 256], F32R, tag="tdm2",
                                 name=f"tdm2_{s}")
                nc.vector.scalar_tensor_tensor(tdm2[:], dut_s[:], float(KEEP),
                                               tdm[:], OP.is_lt, OP.mult)
                tp = accp.tile([128, 512], F32, tag="sp", name=f"tp_{s}")
                nc.tensor.matmul(tp[0:1, 0:256], wtp[:], tdm2[:],
                                 start=True, stop=True)
                traw = tiny.tile([1, 256], F32, tag="traw", name=f"traw_{s}")
                nc.vector.tensor_scalar_min(traw[:], tp[0:1, 0:256], 1.0)
                mn = tiny.tile([1, 1], F32, tag="mn", name=f"mn_{s}")
                nc.vector.tensor_reduce(mn[:], traw[:], AX.X, OP.min)
                flag = tiny.tile([1, 1], F32, tag="flag", name=f"flag_{s}")
                nc.vector.tensor_single_scalar(flag[:], mn[:], 0.1, OP.is_lt)
                delta = tiny.tile([1, 1], F32, tag="delta", name=f"delta_{s}")
                nc.vector.tensor_tensor(delta[:], mn[:], flag[:], OP.mult)
                t2 = tiny.tile([1, 256], F32, tag="t2", name=f"t2_{s}")
                nc.vector.tensor_scalar_sub(t2[:], traw[:], delta[:])
                mx = tiny.tile([1, 1], F32, tag="mx", name=f"mx_{s}")
                nc.vector.tensor_reduce(mx[:], t2[:], AX.X, OP.max)
                r1 = tiny.tile([1, 1], F32, tag="r1", name=f"r1_{s}")
                nc.vector.reciprocal(r1[:], mx[:])
                gtf = tiny.tile([1, 1], F32, tag="gtf", name=f"gtf_{s}")
                nc.vector.tensor_single_scalar(gtf[:], mx[:], 1.0, OP.is_gt)
                fac = tiny.tile([1, 1], F32, tag="fac", name=f"fac_{s}")
                nc.vector.scalar_tensor_tensor(fac[:], r1[:], -1.0, gtf[:],
                                               OP.add, OP.mult)
                t3 = tiny.tile([1, 256], F32, tag="t3", name=f"t3_{s}")
                nc.vector.scalar_tensor_tensor(t3[:], t2[:], fac[:], t2[:],
                                               OP.mult, OP.add)
                t5 = tiny.tile([1, 256], F32R, tag="t5", name=f"t5_{s}")
                if lastt is None:
                    # last_t = 0 and t >= 0 after the shift: only min vs 1
                    nc.vector.tensor_scalar_min(t5[:], t3[:], 1.0)
                else:
                    t4 = tiny.tile([1, 256], F32, tag="t4", name=f"t4_{s}")
                    nc.vector.tensor_tensor(t4[:], t3[:], lastt[:], OP.max)
                    nc.vector.tensor_scalar_min(t5[:], t4[:], 1.0)
                lastt = t5
                nc.sync.dma_start(out_ts[s], t5[:])
                htp = accp.tile([128, 512], F32, tag="sp", name=f"htp_{s}")
                nc.tensor.matmul(htp[0:32, 0:256], wtu[:], t5[:],
                                 start=True, stop=True)
                nc.scalar.activation(cat2[0:32, :], htp[0:32, 0:256],
                                     AF.Copy, bias=btu[:])

            # epilogue: final Hg from last AllReduce, then prob
            ar = cc_out_prev
            arE = tiny.tile([128, 256], F32, tag="arE", name="arE_f")
            nc.sync.dma_start(arE[:], ar[0:128, :])
            rsum = tiny.tile([1, 256], F32, tag="rsum", name="rs_f")
            nc.sync.dma_start(rsum[:], ar[128:129, :])
            recip = tiny.tile([1, 256], F32R, tag="recip", name="recip_f")
            nc.vector.reciprocal(recip[:], rsum[:])
            bc = accp.tile([128, 512], F32, tag="sp", name="bc_f")
            nc.tensor.matmul(bc[:, 0:256], cones[0:1, 0:128], recip[:],
                             start=True, stop=True)
            hg_f = st.tile([128, 256], F32R, tag="hg", name="hg_f")
            nc.vector.tensor_tensor(hg_f[:], arE[:], bc[:, 0:256], OP.mult)
            prp = accp.tile([128, 512], F32, tag="sp", name="prp")
            nc.tensor.matmul(prp[0:1, 0:256], wpva[:], hg_f[:], start=True,
                             stop=False)
            nc.tensor.matmul(prp[0:1, 0:256], wpvb[:], cat2[:], start=False,
                             stop=True)
            prout = tiny.tile([1, 256], F32, tag="prout", name="prout")
            nc.scalar.activation(prout[:], prp[0:1, 0:256], AF.Copy)
            nc.sync.dma_start(out_pr, prout[:])

    nc.compile()
    return nc


def prep_inputs(inputs):
    """Host-side preparation of all per-core DRAM parameters."""
    f32 = np.float32
    W_ih = np.asarray(inputs["W_ih"], f32)
    W_hh = np.asarray(inputs["W_hh"], f32)
    b_ih = np.asarray(inputs["b_ih"], f32)
    b_hh = np.asarray(inputs["b_hh"], f32)
    W_up = np.asarray(inputs["W_up"], f32)
    b_up = np.asarray(inputs["b_up"], f32)
    W_down = np.asarray(inputs["W_down"], f32)
    W_vt = np.asarray(inputs["W_vt"], f32)
    W_prob = np.asarray(inputs["W_prob"], f32)
    Wt_down = np.asarray(inputs["Wt_down"], f32)
    bt_down = np.asarray(inputs["bt_down"], f32)
    Wt_pred = np.asarray(inputs["Wt_pred"], f32)
    Wt_up = np.asarray(inputs["Wt_up"], f32)
    bt_up = np.asarray(inputs["bt_up"], f32)
    gumbel = np.asarray(inputs["gumbel"], f32)
    drop_u = np.asarray(inputs["drop_u"], f32)
    latent = np.asarray(inputs["latent"], f32)
    inputs0 = np.asarray(inputs["inputs0"], f32)

    lat = np.tanh(latent @ np.asarray(inputs["Wc"], f32).T
                  + np.asarray(inputs["bc"], f32))
    s_ = np.tanh(lat @ np.asarray(inputs["Ws"], f32).T
                 + np.asarray(inputs["bs"], f32))
    h0 = np.tanh(s_ @ np.asarray(inputs["Wh"], f32).T
                 + np.asarray(inputs["bh"], f32))
    c0 = np.tanh(s_ @ np.asarray(inputs["Wcc"], f32).T
                 + np.asarray(inputs["bcc"], f32))

    W_ihvt = (W_ih @ W_vt).astype(f32)            # [2048, 160]
    bias_g = (b_ih + b_hh).astype(f32)            # [2048]
    W_pvt = (W_prob @ W_vt).astype(f32)           # [1, 160]

    shared = {
        "whh": fold_h(W_hh.T * 0.5, 4),
        "wiva": np.ascontiguousarray(W_ihvt[:, :128].T),
        "wivb": np.ascontiguousarray(
            np.concatenate([W_ihvt[:, 128:160].T, bias_g[None, :]], axis=0)),
        "wih0": np.ascontiguousarray(W_ih.T),
        "x0t": np.ascontiguousarray(inputs0.T),
        "cones": np.ones((128, 256), f32),
        "wtd": fold_h(Wt_down.T * 0.5, 4),
        "btd": np.ascontiguousarray(bt_down.reshape(32, 1)),
        "wtp": np.ascontiguousarray((Wt_pred / KEEP).T),
        "wtu": np.ascontiguousarray(Wt_up.T),
        "btu": np.ascontiguousarray(bt_up.reshape(32, 1)),
        "wpva": np.ascontiguousarray(W_pvt[:, :128].T),
        "wpvb": np.ascontiguousarray(
            np.concatenate([W_pvt[:, 128:160].T, np.zeros((1, 1), f32)],
                           axis=0)),
        "cat0": np.concatenate([np.zeros((32, 256), f32),
                                np.ones((1, 256), f32)], axis=0),
        "hx0": fold_h(2.0 * h0.T, 4),
        "cs0": fold_h(2.0 * c0.T, 4),
        "dut": np.ascontiguousarray(drop_u.transpose(0, 2, 1)),
    }

    G = np.exp(gumbel + b_up[None, None, :]).astype(f32)       # [16, 256, N]
    in_maps = []
    for c in range(NCORES):
        Wc_up = W_up[c * NLOC:(c + 1) * NLOC, :]
        Wc_dn = W_down[:, c * NLOC:(c + 1) * NLOC]
        Gc = G[:, :, c * NLOC:(c + 1) * NLOC]
        egb = np.ascontiguousarray(
            Gc.reshape(RW, 256, 16, 128).transpose(0, 3, 2, 1)
              .reshape(RW, 128, 4096))
        m = dict(shared)
        m["wup"] = fold_h(Wc_up.T * 0.5, 4)
        m["wdn"] = fold_h(np.ascontiguousarray(Wc_dn.T), 16)
        m["egb"] = egb
        in_maps.append(m)
    return in_maps


def assemble(results):
    """Gather per-core outputs into (rw, ts, prob)."""
    E = np.stack([results[c]["out_e"] for c in range(NCORES)])
    E = E.reshape(NCORES, RW, 128, 16, 256)
    # rw[b, s, c*2048 + t*128 + p] = E[c, s, p, t, b]
    rw = np.ascontiguousarray(E.transpose(4, 1, 0, 3, 2)).reshape(B, RW, N)
    rowsum = rw.sum(axis=2, dtype=np.float64)
    rw = (rw / rowsum[:, :, None]).astype(np.float32)
    ts = np.ascontiguousarray(
        results[0]["out_ts"][:, 0, :].T).reshape(B, RW, 1)
    prob = np.ascontiguousarray(results[0]["out_pr"].T)
    return rw, ts, prob


def get_nc():
    if "nc" not in _cached:
        _cached["nc"] = build()
    return _cached["nc"]


def kernel(**inputs):
    nc = get_nc()
    in_maps = prep_inputs(inputs)
    res = bass_utils.run_bass_kernel_spmd(nc, in_maps,
                                          core_ids=list(range(NCORES)))
    return assemble(res.results)
